# revision 1
# baseline (speedup 1.0000x reference)
"""Trainium2 Bass kernel for a ResNet Bottleneck block (inference).

Reference computation (NCHW, N=128, Cin=Cout=1024, width=256, H=W=14):
    out = relu(bn1(conv1x1(x, w1)))          # 1024 -> 256
    out = relu(bn2(conv3x3(out, w2, pad=1))) # 256 -> 256
    out = bn3(conv1x1(out, w3))              # 256 -> 1024
    y   = relu(out + x)

Strategy:
- Data-parallel: batch 128 sharded as 16 images per NeuronCore (8 cores),
  conv/BN params replicated. One NEFF, SPMD via run_bass_kernel_spmd.
- BN folded on host into per-channel weight scale + bias.
- All convs are matmuls on the TensorEngine with channels on the partition
  (contraction) dim. The 3x3 conv uses a zero-padded 16x16 per-image SBUF
  layout; each of the 9 taps is a shifted-window matmul accumulating in PSUM.
- Compute in bf16 (moving+stationary operands), fp32 PSUM accumulation,
  fp32 output. Residual is added from the bf16 x tiles on the VectorEngine;
  bias+ReLU on the ScalarEngine during PSUM eviction.
"""

import sys

if "/opt/trn_rl_repo" not in sys.path:
    sys.path.insert(0, "/opt/trn_rl_repo")

import numpy as np
import ml_dtypes

import concourse.bass as bass
import concourse.bacc as bacc
import concourse.tile as tile
from concourse import mybir
from concourse.bass_utils import run_bass_kernel_spmd

EPS = 1e-5
NCORES = 8
NLOC = 16          # images per core
C_IN = 1024
WIDTH = 256
C_OUT = 1024
HW = 196           # 14*14
PADHW = 256        # 16*16 zero-padded image
P = 128
KB1 = C_IN // P    # 8 k-blocks for conv1 / residual channel blocks
KB2 = WIDTH // P   # 2 k-blocks for conv2/conv3 input
MB3 = C_OUT // P   # 8 m-blocks for conv3 output
NPAIRS = NLOC // 2  # 8 image pairs; N=392 per matmul
NF = 2 * HW        # 392

BF16 = mybir.dt.bfloat16
F32 = mybir.dt.float32
Relu = mybir.ActivationFunctionType.Relu

_cached = {}


def _build():
    """Build + compile the SPMD NEFF (one core's program). Cached."""
    if "nc" in _cached:
        return _cached["nc"]

    nc = bacc.Bacc("TRN2", target_bir_lowering=False, debug=False,
                   num_devices=NCORES)

    xt_d = nc.dram_tensor("xt", [2, KB1, P, NLOC * HW // 2], BF16,
                          kind="ExternalInput")
    # weights pre-arranged host-side as exact SBUF images (partition-major),
    # so each loads with ONE DMA at max descriptor size
    w1_d = nc.dram_tensor("w1t", [P, KB1 * WIDTH], BF16, kind="ExternalInput")
    w2_d = nc.dram_tensor("w2t", [P, 9 * KB2 * WIDTH], BF16,
                          kind="ExternalInput")
    w3_d = nc.dram_tensor("w3t", [P, KB2 * C_OUT], BF16, kind="ExternalInput")
    b_d = nc.dram_tensor("biases", [P, 2 * KB2 + MB3], F32,
                         kind="ExternalInput")
    id_d = nc.dram_tensor("ident", [P, P], BF16, kind="ExternalInput")
    y_d = nc.dram_tensor("y", [MB3, P, NLOC * HW], BF16, kind="ExternalOutput")

    with tile.TileContext(nc) as tc:
        _emit(tc, nc, xt_d, w1_d, w2_d, w3_d, b_d, id_d, y_d)

    nc.compile()
    _cached["nc"] = nc
    return nc


def _emit(tc, nc, xt_d, w1_d, w2_d, w3_d, b_d, id_d, y_d):
    """PE-density-oriented emission.

    - Accumulation chains into the SAME PSUM bank serialize at the matmul
      latency (~329ns for N=392); chains interleaved across banks pipeline
      at the issue rate (~169ns). So every phase runs its contraction loop
      OUTER over 8 concurrently-open PSUM groups (8 banks), group index
      innermost so consecutive matmuls target different banks.
    - DMA *issue* is ~0.6us per dma_start on the issuing engine: inputs are
      consolidated into 13 DMAs on Sync; outputs are staged into [P, 3136]
      tiles and written with 2 DMAs per m-block issued from GpSimd.
    - The residual add runs on the PE as an identity-weight matmul appended
      to each conv3 accumulation group, so eviction is a single
      relu(psum+bias) op, alternating VectorE/ScalarE.
    """
    import contextlib

    Alu = mybir.AluOpType

    def evict_relu_bias(dst, src, bias_ap, on_vector):
        # dst = relu(src + bias)
        if on_vector:
            nc.vector.tensor_scalar(dst, src, bias_ap, 0.0, Alu.add, Alu.max)
        else:
            nc.scalar.activation(dst, src, Relu, bias=bias_ap)

    with contextlib.ExitStack() as ctx:
        const = ctx.enter_context(tc.tile_pool(name="const", bufs=1))
        xpool = ctx.enter_context(tc.tile_pool(name="xpool", bufs=1))
        opool = ctx.enter_context(tc.tile_pool(name="opool", bufs=1))
        psp = ctx.enter_context(tc.tile_pool(name="psp", bufs=8, space="PSUM"))
        evp = ctx.enter_context(tc.tile_pool(name="evp", bufs=2))

        # ---- Loads, in consumption order, one DMA each -------------------
        # DMA engines stripe packets fairly across ALL active transfers, so
        # concurrent DMAs all complete near the end of the aggregate window.
        # Chain the x loads (depth 3) so early tiles finish early and conv1
        # can consume them as they land.
        from concourse.tile import add_dep_helper

        HNF = 4 * NF
        xsb = xpool.tile([P, KB1 * NLOC * HW], BF16, name="xsb", tag="xsb")
        x_tiles = [xsb[:, k * NLOC * HW:(k + 1) * NLOC * HW]
                   for k in range(KB1)]
        # x dram is half-major; load each half as 4 two-k-tile DMAs (784KB):
        # larger transfers saturate DMA bandwidth with fewer chain links.
        xv = xsb[:].rearrange("p (k h c) -> p k h c", k=KB1, h=2)
        x_dmas = []
        for half in range(2):
            eng = nc.sync if half == 0 else nc.gpsimd
            for j in range(KB1 // 2):
                dst = xv[:, 2 * j:2 * j + 2, half, :]
                s = (xt_d.ap()[half][2 * j:2 * j + 2]
                     .rearrange("k p c -> p k c"))
                i = eng.dma_start(dst, s)
                n = len(x_dmas)
                if n >= 2:
                    add_dep_helper(i.ins, x_dmas[n - 2],
                                   reason="x load pacing")
                x_dmas.append(i.ins)

        w1sb = const.tile([P, KB1 * WIDTH], BF16, name="w1sb", tag="w1sb")
        nc.scalar.dma_start(w1sb[:], w1_d.ap())
        w1_t = [w1sb[:, k * WIDTH:(k + 1) * WIDTH] for k in range(KB1)]

        ball = const.tile([P, 2 * KB2 + MB3], F32, name="ball", tag="ball")
        i = nc.scalar.dma_start(ball[:], b_d.ap())
        add_dep_helper(i.ins, x_dmas[0], reason="bias after early x")
        b1_t = ball[:, 0:KB2]
        b2_t = ball[:, KB2:2 * KB2]
        b3_t = ball[:, 2 * KB2:]

        w2sb = const.tile([P, 9 * KB2 * WIDTH], BF16, name="w2sb", tag="w2sb")
        i = nc.gpsimd.dma_start(w2sb[:], w2_d.ap())
        add_dep_helper(i.ins, x_dmas[5], reason="w2 near end of x")
        w2_t = [[w2sb[:, (tap * KB2 + k) * WIDTH:(tap * KB2 + k + 1) * WIDTH]
                 for k in range(KB2)] for tap in range(9)]

        w3sb = const.tile([P, KB2 * C_OUT], BF16, name="w3sb", tag="w3sb")
        i = nc.gpsimd.dma_start(w3sb[:], w3_d.ap())
        add_dep_helper(i.ins, x_dmas[7], reason="w3 after x")
        w3_t = [w3sb[:, k * C_OUT:(k + 1) * C_OUT] for k in range(KB2)]

        id_t = const.tile([P, P], BF16, name="id_t", tag="id_t")
        i = nc.gpsimd.dma_start(id_t[:], id_d.ap())
        add_dep_helper(i.ins, x_dmas[7], reason="ident after x")

        # PE warm-up: the HAM clock gate needs ~3.4us of sustained PE
        # activity to lift the PE from 1.2 to 2.4 GHz. Run dummy matmuls on
        # a scratch tile while the first x DMAs are still in flight.
        scratch = const.tile([P, 512], BF16, name="scratch", tag="scratch")
        nc.gpsimd.memset(scratch[:], 0.0)
        warm_ps = psp.tile([P, 512], F32, name="warm_ps", tag="ps")
        for _ in range(8):
            nc.tensor.matmul(warm_ps[:], scratch[:, 0:P], scratch[:],
                             start=True, stop=True)

        # Zero-padded conv1 output: per image a 16x16 field, payload at
        # rows/cols 1..14. Layout [P, NLOC*256].
        out1 = []
        for m in range(KB2):
            t = opool.tile([P, NLOC * PADHW], BF16, name=f"out1_{m}",
                           tag=f"out1_{m}")
            nc.vector.memset(t[:], 0.0)
            out1.append(t)

        out2 = []
        for m in range(KB2):
            t = opool.tile([P, NLOC * HW], BF16, name=f"out2_{m}",
                           tag=f"out2_{m}")
            out2.append(t)

        def pad_view(k, np_):
            return (out1[k][:, np_ * 2 * PADHW:(np_ + 1) * 2 * PADHW]
                    .rearrange("p (i r c) -> p i r c", i=2, r=16, c=16))

        # ---- conv1 (1x1, 1024->256) + bias + relu -> padded out1 --------
        # Per np-half: 8 open groups (4 npairs x 2 m), contraction k outer.
        for half in range(2):
            nps = [half * 4 + j for j in range(4)]
            grp = {}
            for np_ in nps:
                for m in range(KB2):
                    ps = psp.tile([P, NF], F32, name=f"ps1_{np_}_{m}",
                                  tag="ps")
                    grp[(np_, m)] = ps
            for k in range(KB1):
                for m in range(KB2):
                    for np_ in nps:
                        nc.tensor.matmul(
                            grp[(np_, m)][:],
                            w1_t[k][:, m * P:(m + 1) * P],
                            x_tiles[k][:, np_ * NF:(np_ + 1) * NF],
                            start=(k == 0), stop=(k == KB1 - 1),
                        )
            for np_ in nps:
                for m in range(KB2):
                    dst = pad_view(m, np_)[:, :, 1:15, 1:15]
                    src = (grp[(np_, m)][:]
                           .rearrange("p (i r c) -> p i r c", i=2, r=14, c=14))
                    evict_relu_bias(dst, src, b1_t[:, m:m + 1],
                                    on_vector=(np_ % 2 == 1))

        # ---- conv2 (3x3, 256->256, pad 1) + bias + relu -> out2 ----------
        # Per np-half: 8 open groups, contraction (k, dy, dx) outer.
        for half in range(2):
            nps = [half * 4 + j for j in range(4)]
            grp = {}
            for np_ in nps:
                for m in range(KB2):
                    grp[(np_, m)] = psp.tile([P, NF], F32,
                                             name=f"ps2_{np_}_{m}", tag="ps")
            for idx, (k, dy, dx) in enumerate(
                    (k, dy, dx) for k in range(KB2)
                    for dy in range(3) for dx in range(3)):
                for m in range(KB2):
                    for np_ in nps:
                        rhs = pad_view(k, np_)[:, :, dy:dy + 14, dx:dx + 14]
                        nc.tensor.matmul(
                            grp[(np_, m)][:]
                            .rearrange("p (i r c) -> p i r c", i=2, r=14, c=14),
                            w2_t[dy * 3 + dx][k][:, m * P:(m + 1) * P],
                            rhs,
                            start=(idx == 0), stop=(idx == 17),
                        )
            for np_ in nps:
                for m in range(KB2):
                    evict_relu_bias(out2[m][:, np_ * NF:(np_ + 1) * NF],
                                    grp[(np_, m)][:], b2_t[:, m:m + 1],
                                    on_vector=(np_ % 2 == 1))

        # ---- conv3 (1x1, 256->1024) + bias + residual + relu -> y --------
        # Per m: 8 open groups (npairs), contraction k outer. The residual
        # lands in PSUM via an identity-weight matmul closing most groups;
        # two groups per pass take the DVE/ACT path instead to shave PE
        # work (DVE stt computes (psum+bias)+x, ACT applies relu). The last
        # pass stays all-PE so its eviction tail is a single op per group.
        for m in range(MB3):
            bgrps = {5, 6, 7} if m < MB3 - 1 else set()
            grp = {}
            for np_ in range(NPAIRS):
                grp[np_] = psp.tile([P, NF], F32, name=f"ps3_{np_}", tag="ps")
            for k in range(KB2):
                for np_ in range(NPAIRS):
                    nc.tensor.matmul(
                        grp[np_][:],
                        w3_t[k][:, m * P:(m + 1) * P],
                        out2[k][:, np_ * NF:(np_ + 1) * NF],
                        start=(k == 0), stop=(k == KB2 - 1 and np_ in bgrps),
                    )
            for np_ in range(NPAIRS):
                if np_ not in bgrps:
                    nc.tensor.matmul(
                        grp[np_][:], id_t[:],
                        x_tiles[m][:, np_ * NF:(np_ + 1) * NF],
                        start=False, stop=True,
                    )
            ystage = evp.tile([P, NLOC * HW], BF16, name="ystage",
                              tag="ystage", bufs=3)
            for np_ in range(NPAIRS):
                dst = ystage[:, np_ * NF:(np_ + 1) * NF]
                if np_ in bgrps:
                    tsum = evp.tile([P, NF], F32, name="tsum", tag="tsum",
                                    bufs=4)
                    nc.vector.scalar_tensor_tensor(
                        tsum[:], grp[np_][:], b3_t[:, m:m + 1],
                        x_tiles[m][:, np_ * NF:(np_ + 1) * NF],
                        Alu.add, Alu.add)
                    nc.scalar.activation(dst, tsum[:], Relu, bias=0.0)
                else:
                    evict_relu_bias(dst, grp[np_][:], b3_t[:, m:m + 1],
                                    on_vector=(np_ % 2 == 1))
            nchunk = 4 if m == MB3 - 1 else 2
            CNF = NLOC * HW // nchunk
            for c in range(nchunk):
                nc.sync.dma_start(y_d.ap()[m][:, c * CNF:(c + 1) * CNF],
                                  ystage[:, c * CNF:(c + 1) * CNF])


def _prep(x, w1, g1, b1, m1, v1, w2, g2, b2, m2, v2, w3, g3, b3, m3, v3):
    """Host-side: fold BN, transpose weights to lhsT layouts, shard x."""
    def fold(w, g, b, m, v):
        scale = (g.astype(np.float64) / np.sqrt(v.astype(np.float64) + EPS))
        bias = b.astype(np.float64) - m.astype(np.float64) * scale
        wf = w.astype(np.float64) * scale.reshape(-1, *([1] * (w.ndim - 1)))
        return wf.astype(np.float32), bias.astype(np.float32)

    w1f, bias1 = fold(w1, g1, b1, m1, v1)   # [256,1024,1,1]
    w2f, bias2 = fold(w2, g2, b2, m2, v2)   # [256,256,3,3]
    w3f, bias3 = fold(w3, g3, b3, m3, v3)   # [1024,256,1,1]

    bf = ml_dtypes.bfloat16
    # lhsT SBUF images [P(=ci within kblock), ...]:
    # w1: [k, p, co] -> [p, (k co)]
    w1t = np.ascontiguousarray(
        w1f[:, :, 0, 0].T.reshape(KB1, P, WIDTH).transpose(1, 0, 2)
        .reshape(P, KB1 * WIDTH)).astype(bf)
    # w2: [tap, k, p, co] -> [p, (tap k co)], tap = dy*3+dx
    w2t = np.ascontiguousarray(
        w2f.transpose(2, 3, 1, 0).reshape(9 * KB2, P, WIDTH)
        .transpose(1, 0, 2).reshape(P, 9 * KB2 * WIDTH)).astype(bf)
    # w3: [k, p, co] -> [p, (k co)]
    w3t = np.ascontiguousarray(
        w3f[:, :, 0, 0].T.reshape(KB2, P, C_OUT).transpose(1, 0, 2)
        .reshape(P, KB2 * C_OUT)).astype(bf)

    b1h = bias1.reshape(KB2, P).T                          # [P, 2]
    b2h = bias2.reshape(KB2, P).T                          # [P, 2]
    b3h = bias3.reshape(MB3, P).T                          # [P, 8]
    ball = np.ascontiguousarray(
        np.concatenate([b1h, b2h, b3h], axis=1), dtype=np.float32)

    # x: [128, 1024, 14, 14] -> per core [2(half), KB1, P, NLOC*HW/2] bf16
    xs = (x.reshape(NCORES, NLOC, KB1, P, HW)
          .transpose(0, 2, 3, 1, 4)
          .reshape(NCORES, KB1, P, NLOC * HW)).astype(bf)
    H = NLOC * HW // 2
    xs = np.stack((xs[..., :H], xs[..., H:]), axis=1)  # [cores,2,KB1,P,H]

    common = {"w1t": w1t, "w2t": w2t, "w3t": w3t,
              "biases": ball, "ident": np.eye(P, dtype=np.float32).astype(bf)}
    in_maps = [dict(common, xt=np.ascontiguousarray(xs[i]))
               for i in range(NCORES)]
    return in_maps


def kernel(**inputs):
    inputs = {k: np.asarray(v) for k, v in inputs.items()}
    in_maps = _prep(**inputs)
    nc = _build()
    res = run_bass_kernel_spmd(nc, in_maps, core_ids=list(range(NCORES)))

    y = np.empty((NCORES * NLOC, C_OUT, 14, 14), dtype=np.float32)
    for i in range(NCORES):
        r = np.asarray(res.results[i]["y"], dtype=np.float32)  # [MB3,P,N*HW]
        r = (r.reshape(MB3, P, NLOC, HW)
             .transpose(2, 0, 1, 3)
             .reshape(NLOC, C_OUT, 14, 14))
        y[i * NLOC:(i + 1) * NLOC] = r
    return y



# revision 3
# speedup vs baseline: 1.4104x; 1.4104x over previous
"""Trainium2 Bass kernel for a ResNet Bottleneck block (inference).

Reference computation (NCHW, N=128, Cin=Cout=1024, width=256, H=W=14):
    out = relu(bn1(conv1x1(x, w1)))          # 1024 -> 256
    out = relu(bn2(conv3x3(out, w2, pad=1))) # 256 -> 256
    out = bn3(conv1x1(out, w3))              # 256 -> 1024
    y   = relu(out + x)

Strategy (fp8 DoubleRow):
- Data-parallel: batch 128 sharded as 16 images per NeuronCore (8 cores).
- All convs run as fp8e4 (e4m3) DoubleRow matmuls: 2 fp8 weights/cell double
  the effective contraction to 256/matmul (~1.5x bf16 TFLOP/s at free-dim
  >=196). PSUM accumulates fp32, so precision loss is only operand
  quantization; measured end-to-end rel err ~8e-3 (tol 2e-2).
- BN folded on host into weight scale + bias. Weights are rescaled by
  powers of two (s1=32, s2=2, s3=16) to lift their ~0.02 std out of
  e4m3's subnormal range; ReLU's positive homogeneity carries the scale
  through layers, evictions add correspondingly scaled biases, and the
  host divides the final bf16 output by s1*s2*s3 = 1024.
- conv2 (3x3, pad 1) uses a zero-padded 16x16 per-image SBUF layout; each
  of the 9 taps is one shifted-window DoubleRow matmul per image (moving
  AP [p, ktile=2, row14, col14]; matmul APs allow at most 3 free dims,
  so images can't be paired here).
- Residual + bias3 are folded host-side into xr = 1024*(x + b3), bf16.
  Half the conv3 groups add it on the PE (bf16 identity matmul appended
  to the fp8 accumulation group), half on DVE (tensor_tensor add) with
  the ReLU on ACT, balancing PE vs eviction-engine load.
- conv2+conv3 are pipelined per 4-image "super" block so conv3 evictions
  overlap the next block's conv2 matmuls.
"""

import sys

if "/opt/trn_rl_repo" not in sys.path:
    sys.path.insert(0, "/opt/trn_rl_repo")

import numpy as np
import ml_dtypes

import concourse.bass as bass
import concourse.bacc as bacc
import concourse.tile as tile
from concourse import mybir
from concourse.bass_utils import run_bass_kernel_spmd

EPS = 1e-5
NCORES = 8
NLOC = 16          # images per core
P = 128
C_IN = 1024
WIDTH = 256
C_OUT = 1024
HW = 196           # 14*14
JB = 4             # conv1 contraction double-blocks (1024 = 4*256)
MB3 = 8            # conv3 output 128-blocks
NPAIRS = 8         # image pairs per core
NF = 2 * HW        # 392

S1, S2, S3 = 32.0, 2.0, 16.0
STOT = S1 * S2 * S3            # 1024; fp8 activation scales: out1 32x, out2 64x

BF16 = mybir.dt.bfloat16
F32 = mybir.dt.float32
FP8 = mybir.dt.float8e4
DR = mybir.MatmulPerfMode.DoubleRow
Relu = mybir.ActivationFunctionType.Relu

_cached = {}


def _build():
    """Build + compile the SPMD NEFF (one core's program). Cached."""
    if "nc" in _cached:
        return _cached["nc"]

    nc = bacc.Bacc("TRN2", target_bir_lowering=False, debug=False,
                   num_devices=NCORES)

    # DRAM layouts are exact SBUF images (partition-major), packed host-side.
    x8_d = nc.dram_tensor("x8", [2, P, JB * 2 * 8 * HW], FP8,
                          kind="ExternalInput")
    xr_d = nc.dram_tensor("xr", [NPAIRS, P, MB3 * NF], BF16,
                          kind="ExternalInput")
    w1_d = nc.dram_tensor("w1t", [P, JB * 2 * WIDTH], FP8,
                          kind="ExternalInput")
    w2_d = nc.dram_tensor("w2t", [P, 9 * 2 * WIDTH], FP8,
                          kind="ExternalInput")
    w3_d = nc.dram_tensor("w3t", [P, 2 * C_OUT], FP8, kind="ExternalInput")
    b_d = nc.dram_tensor("biases", [P, 4], F32, kind="ExternalInput")
    id_d = nc.dram_tensor("ident", [P, P], BF16, kind="ExternalInput")
    y_d = nc.dram_tensor("y", [NPAIRS, P, MB3 * NF], BF16,
                         kind="ExternalOutput")

    with tile.TileContext(nc) as tc:
        _emit(tc, nc, x8_d, xr_d, w1_d, w2_d, w3_d, b_d, id_d, y_d)

    nc.compile()
    _cached["nc"] = nc
    return nc


def _emit(tc, nc, x8_d, xr_d, w1_d, w2_d, w3_d, b_d, id_d, y_d):
    import contextlib
    from concourse.tile import add_dep_helper

    Alu = mybir.AluOpType
    HF = 8 * HW                        # 1568, one half's free extent

    with contextlib.ExitStack() as ctx:
        const = ctx.enter_context(tc.tile_pool(name="const", bufs=1))
        xpool = ctx.enter_context(tc.tile_pool(name="xpool", bufs=1))
        opool = ctx.enter_context(tc.tile_pool(name="opool", bufs=1))
        psp = ctx.enter_context(tc.tile_pool(name="psp", bufs=8, space="PSUM"))
        evp = ctx.enter_context(tc.tile_pool(name="evp", bufs=2))

        # ---- Loads, in consumption order -------------------------------
        x8sb = xpool.tile([P, JB, 2, NLOC * HW], FP8, name="x8sb", tag="x8sb")
        x8v = x8sb[:].rearrange("p j k f -> p (j k) f")
        x_dmas = []
        for half in range(2):
            eng = nc.sync if half == 0 else nc.gpsimd
            src = x8_d.ap()[half].rearrange("p (b f) -> p b f", b=2 * JB)
            for jh in range(2):
                dst = x8v[:, 4 * jh:4 * jh + 4, half * HF:(half + 1) * HF]
                i = eng.dma_start(dst, src[:, 4 * jh:4 * jh + 4, :])
                n = len(x_dmas)
                if n >= 2:
                    add_dep_helper(i.ins, x_dmas[n - 2], reason="x pacing")
                x_dmas.append(i.ins)

        w1sb = const.tile([P, JB, 2, WIDTH], FP8, name="w1sb", tag="w1sb")
        i = nc.scalar.dma_start(w1sb[:].rearrange("p a k c -> p (a k c)"),
                                w1_d.ap())
        add_dep_helper(i.ins, x_dmas[0], reason="w1 after first x")

        ball = const.tile([P, 4], F32, name="ball", tag="ball")
        i = nc.scalar.dma_start(ball[:], b_d.ap())
        add_dep_helper(i.ins, x_dmas[0], reason="bias after first x")

        w2sb = const.tile([P, 9, 2, WIDTH], FP8, name="w2sb", tag="w2sb")
        i = nc.gpsimd.dma_start(w2sb[:].rearrange("p t k c -> p (t k c)"),
                                w2_d.ap())
        add_dep_helper(i.ins, x_dmas[2], reason="w2 during x h1")

        w3sb = const.tile([P, 2, C_OUT], FP8, name="w3sb", tag="w3sb")
        i = nc.gpsimd.dma_start(w3sb[:].rearrange("p k c -> p (k c)"),
                                w3_d.ap())
        add_dep_helper(i.ins, x_dmas[3], reason="w3 after x")

        id_t = const.tile([P, P], BF16, name="id_t", tag="id_t")
        i = nc.gpsimd.dma_start(id_t[:], id_d.ap())
        add_dep_helper(i.ins, x_dmas[3], reason="ident after x")

        xrsb = xpool.tile([P, NPAIRS, MB3, NF], BF16, name="xrsb", tag="xrsb")
        xr_dmas = []
        for np_ in range(NPAIRS):
            dst = xrsb[:, np_, :, :].rearrange("p m f -> p (m f)")
            i = nc.gpsimd.dma_start(dst, xr_d.ap()[np_])
            prev = xr_dmas[-1] if xr_dmas else x_dmas[3]
            add_dep_helper(i.ins, prev, reason="xr chain")
            xr_dmas.append(i.ins)

        # PE warm-up: lift the HAM clock gate (needs ~3.4us of PE activity)
        # while the first x DMAs are still in flight.
        scratch = const.tile([P, 512], BF16, name="scratch", tag="scratch")
        nc.gpsimd.memset(scratch[:], 0.0)
        warm_ps = psp.tile([P, 512], F32, name="warm_ps", tag="ps")
        for _ in range(8):
            nc.tensor.matmul(warm_ps[:], scratch[:, 0:P], scratch[:],
                             start=True, stop=True)

        # Zero-padded conv1 output: per image a 16x16 field per 128-block,
        # payload at rows/cols 1..14.
        out1 = opool.tile([P, 2, NLOC, 16, 16], FP8, name="out1", tag="out1")
        o1flat = out1[:].rearrange("p k i r c -> p k (i r c)")
        for half in range(2):
            nc.vector.memset(o1flat[:, :, half * 2048:(half + 1) * 2048], 0.0)

        out2 = opool.tile([P, 2, NLOC * HW], FP8, name="out2", tag="out2")

        tog = [0]

        def evict_relu_bias(dst, src, bias_ap):
            # dst = relu(src + bias), alternating DVE / ACT
            tog[0] ^= 1
            if tog[0]:
                nc.vector.tensor_scalar(dst, src, bias_ap, 0.0, Alu.add,
                                        Alu.max)
            else:
                nc.scalar.activation(dst, src, Relu, bias=bias_ap)

        # ---- conv1 (1x1, 1024->256) + bias + relu -> padded out1 --------
        # Per half: 8 open groups (4 pairs x 2 out-blocks), contraction j
        # outer, groups inner so consecutive matmuls hit different banks.
        for half in range(2):
            nls = range(4)
            grp = {(nl, mo): psp.tile([P, NF], F32, name=f"ps1_{nl}_{mo}",
                                      tag="ps")
                   for nl in nls for mo in range(2)}
            for j in range(JB):
                for mo in range(2):
                    w_ap = w1sb[:, j, :, mo * P:(mo + 1) * P]
                    for nl in nls:
                        np_ = 4 * half + nl
                        nc.tensor.matmul(
                            grp[(nl, mo)][:], w_ap,
                            x8sb[:, j, :, np_ * NF:(np_ + 1) * NF],
                            start=(j == 0), stop=(j == JB - 1),
                            perf_mode=DR)
            for nl in nls:
                np_ = 4 * half + nl
                for mo in range(2):
                    dst = out1[:, mo, 2 * np_:2 * np_ + 2, 1:15, 1:15]
                    src = (grp[(nl, mo)][:]
                           .rearrange("p (i r c) -> p i r c", i=2, r=14))
                    evict_relu_bias(dst, src, ball[:, mo:mo + 1])

        # ---- conv2 + conv3 pipelined per 4-image super-block ------------
        for s in range(4):
            # conv2 (3x3, 256->256, pad 1): 8 groups (4 imgs x 2 out-blocks),
            # contraction tap outer. Per-image matmuls (N=196): the windowed
            # moving AP [p, kt, r, c] is at the 3-free-dim ISA limit.
            g2 = {(ii, mo): psp.tile([P, HW], F32, name=f"ps2_{ii}_{mo}",
                                     tag="ps")
                  for ii in range(4) for mo in range(2)}
            for tap in range(9):
                dy, dx = tap // 3, tap % 3
                for mo in range(2):
                    w_ap = w2sb[:, tap, :, mo * P:(mo + 1) * P]
                    for ii in range(4):
                        img = 4 * s + ii
                        nc.tensor.matmul(
                            g2[(ii, mo)][:].rearrange("p (r c) -> p r c",
                                                      r=14),
                            w_ap,
                            out1[:, :, img, dy:dy + 14, dx:dx + 14],
                            start=(tap == 0), stop=(tap == 8),
                            perf_mode=DR)
            for ii in range(4):
                img = 4 * s + ii
                for mo in range(2):
                    evict_relu_bias(out2[:, mo, img * HW:(img + 1) * HW],
                                    g2[(ii, mo)][:], ball[:, 2 + mo:3 + mo])

            # conv3 (1x1, 256->1024) + residual + relu, two waves of 8
            # groups (4 m-blocks x 2 pairs). Half the groups take the
            # residual as a bf16 identity matmul on the PE; half on DVE.
            yst = {nl: evp.tile([P, MB3 * NF], BF16, name=f"yst{nl}",
                                tag="yst", bufs=3) for nl in range(2)}
            for wave in range(2):
                g3 = {}
                on_pe = {}
                for mi in range(4):
                    m = 4 * wave + mi
                    w_ap = w3sb[:, :, m * P:(m + 1) * P]
                    for nl in range(2):
                        np_ = 2 * s + nl
                        g = psp.tile([P, NF], F32, name=f"ps3_{mi}_{nl}",
                                     tag="ps")
                        g3[(mi, nl)] = g
                        on_pe[(mi, nl)] = (mi + nl) % 2 == 0
                        nc.tensor.matmul(
                            g[:], w_ap,
                            out2[:, :, np_ * NF:(np_ + 1) * NF],
                            start=True, stop=not on_pe[(mi, nl)],
                            perf_mode=DR)
                for mi in range(4):
                    m = 4 * wave + mi
                    for nl in range(2):
                        if not on_pe[(mi, nl)]:
                            continue
                        np_ = 2 * s + nl
                        nc.tensor.matmul(
                            g3[(mi, nl)][:], id_t[:], xrsb[:, np_, m, :],
                            start=False, stop=True)
                for mi in range(4):
                    m = 4 * wave + mi
                    for nl in range(2):
                        np_ = 2 * s + nl
                        dst = yst[nl][:, m * NF:(m + 1) * NF]
                        if on_pe[(mi, nl)]:
                            tog[0] ^= 1
                            if tog[0]:
                                nc.vector.tensor_scalar_max(
                                    dst, g3[(mi, nl)][:], 0.0)
                            else:
                                nc.scalar.activation(dst, g3[(mi, nl)][:],
                                                     Relu, bias=0.0)
                        else:
                            ts = evp.tile([P, NF], F32, name="tsum",
                                          tag="tsum", bufs=4)
                            nc.vector.tensor_tensor(
                                ts[:], g3[(mi, nl)][:], xrsb[:, np_, m, :],
                                Alu.add)
                            nc.scalar.activation(dst, ts[:], Relu, bias=0.0)
            for nl in range(2):
                np_ = 2 * s + nl
                nc.sync.dma_start(y_d.ap()[np_], yst[nl][:])


def _prep(x, w1, g1, b1, m1, v1, w2, g2, b2, m2, v2, w3, g3, b3, m3, v3):
    """Host-side: fold BN, rescale + quantize to fp8, pack SBUF images."""
    def fold(w, g, b, m, v):
        scale = (g.astype(np.float64) / np.sqrt(v.astype(np.float64) + EPS))
        bias = b.astype(np.float64) - m.astype(np.float64) * scale
        wf = w.astype(np.float64) * scale.reshape(-1, *([1] * (w.ndim - 1)))
        return wf.astype(np.float32), bias.astype(np.float32)

    w1f, bias1 = fold(w1, g1, b1, m1, v1)   # [256,1024,1,1]
    w2f, bias2 = fold(w2, g2, b2, m2, v2)   # [256,256,3,3]
    w3f, bias3 = fold(w3, g3, b3, m3, v3)   # [1024,256,1,1]

    bf = ml_dtypes.bfloat16
    e4 = ml_dtypes.float8_e4m3

    def q8(a):
        return np.clip(a, -240.0, 240.0).astype(e4)

    # lhsT SBUF images [p_in, ..., ktile, co]:
    w1t = q8(np.ascontiguousarray(
        (w1f[:, :, 0, 0] * S1).T.reshape(JB, 2, P, WIDTH)
        .transpose(2, 0, 1, 3).reshape(P, JB * 2 * WIDTH)))
    w2t = q8(np.ascontiguousarray(
        (w2f * S2).transpose(2, 3, 1, 0).reshape(3, 3, 2, P, WIDTH)
        .transpose(3, 0, 1, 2, 4).reshape(P, 9 * 2 * WIDTH)))
    w3t = q8(np.ascontiguousarray(
        (w3f[:, :, 0, 0] * S3).T.reshape(2, P, C_OUT)
        .transpose(1, 0, 2).reshape(P, 2 * C_OUT)))

    b1h = (bias1 * S1).reshape(2, P).T                    # [P, 2]
    b2h = (bias2 * S1 * S2).reshape(2, P).T               # [P, 2] (64x)
    ball = np.ascontiguousarray(
        np.concatenate([b1h, b2h], axis=1), dtype=np.float32)

    # x8: conv1 moving operand, [core][half, P, (j, kt, img8, hw)] fp8
    xs = (x.reshape(NCORES, 2, 8, JB, 2, P, HW)
          .transpose(0, 1, 5, 3, 4, 2, 6)
          .reshape(NCORES, 2, P, JB * 2 * 8 * HW))
    x8 = q8(xs)

    # xr: residual + bias3, pre-scaled: STOT*(x + b3), np-major bf16
    r = x.reshape(NCORES, NLOC, C_OUT, HW) + bias3[None, None, :, None]
    xr = ((r * STOT)
          .reshape(NCORES, NPAIRS, 2, MB3, P, HW)
          .transpose(0, 1, 4, 3, 2, 5)
          .reshape(NCORES, NPAIRS, P, MB3 * NF)).astype(bf)

    common = {"w1t": w1t, "w2t": w2t, "w3t": w3t, "biases": ball,
              "ident": np.eye(P, dtype=np.float32).astype(bf)}
    in_maps = [dict(common, x8=np.ascontiguousarray(x8[i]),
                    xr=np.ascontiguousarray(xr[i]))
               for i in range(NCORES)]
    return in_maps


def kernel(**inputs):
    inputs = {k: np.asarray(v) for k, v in inputs.items()}
    in_maps = _prep(**inputs)
    nc = _build()
    res = run_bass_kernel_spmd(nc, in_maps, core_ids=list(range(NCORES)))

    y = np.empty((NCORES * NLOC, C_OUT, 14, 14), dtype=np.float32)
    for i in range(NCORES):
        r = np.asarray(res.results[i]["y"], dtype=np.float32) / STOT
        r = (r.reshape(NPAIRS, P, MB3, 2, HW)
             .transpose(0, 3, 2, 1, 4)
             .reshape(NLOC, C_OUT, 14, 14))
        y[i * NLOC:(i + 1) * NLOC] = r
    return y
